# revision 8
# baseline (speedup 1.0000x reference)
"""Trainium2 Bass kernel for batched multi-head attention with additive
probability bias and boolean mask, returning both the attention output and
the softmax probabilities.

  reference:
    s = (q @ k^T) / sqrt(D) + probs[:, None] * alpha
    s = where(mask == 0, -inf, s)
    p = softmax(s, axis=-1)
    out = p @ v           ->  returns (out, p)

Sharding: batch*head across 8 cores (4 heads / core, each core one batch),
so mask/probs shard along batch and there is no cross-device communication.

Per-core device kernel (natural layout, scores tile = [128 q x 2048 k]):
  - scores accumulate in PSUM: identity-matmul injects the combined bias
    (probs + BIG*(mask-1), pre-scaled so ACT's scale folds 1/sqrt(D) and
    alpha), then q@k^T accumulates on top in float32r (full PE rate).
  - ACT computes exp(scale * psum) with accum_out giving the row sums for
    free; VectorE reciprocal + tensor_scalar normalizes.
  - P^T for the p@v matmul comes from PE transposes of a bf16 copy of P;
    p@v runs in bf16 (fp32 PSUM accumulation).
"""

import sys
import math
from contextlib import ExitStack

if "/opt/trn_rl_repo" not in sys.path:
    sys.path.insert(0, "/opt/trn_rl_repo")

import numpy as np

import concourse.bass as bass
import concourse.mybir as mybir
import concourse.tile as tile
from concourse.bass_utils import run_bass_kernel_spmd
from concourse.masks import make_identity
from concourse.vector_clock import ScopedClock

P = 128
B, H, L, D = 2, 16, 2048, 128
N_CORES = 8
HPC = (B * H) // N_CORES  # heads per core
SCALE = 1.0 / math.sqrt(D)
BIG = 1.0e30

F32 = mybir.dt.float32
F32R = mybir.dt.float32r
BF16 = mybir.dt.bfloat16
I32 = mybir.dt.int32


class PatchedTileContext(tile.TileContext):
    """This walrus build rejects >1 sem wait on a TPB_CTRL instruction;
    split the exit drain's waits across a chain of single-wait NOPs."""

    MAX_WAITS = 1

    def _drain_and_barrier(self, tick_clock, wait_clock):
        probe = self.nc.sync.nop(nofuse=True)
        wait_clock.add_sem_waits(
            probe.ins, ScopedClock({None: tick_clock.global_clock})
        )
        si = probe.ins.sync_info
        waits = list(si.on_wait) if si is not None and si.on_wait else []
        if si is not None:
            si.on_wait = waits[: self.MAX_WAITS]
        rest = waits[self.MAX_WAITS :]
        while rest:
            n2 = self.nc.sync.nop(nofuse=True)
            n2.ins.sync_info = mybir.SyncInfo(
                on_update=[], on_wait=rest[: self.MAX_WAITS]
            )
            rest = rest[self.MAX_WAITS :]
        self.nc.sync.drain()
        self.nc.all_engine_barrier()
        assert self.sems is not None
        popped = self.nc._tile_sem_poison_stack.pop()
        assert popped is self._sem_poison
        self.nc.clear_and_free_semaphores(list(self.sems.allocated().values()))
        self.nc.all_engine_barrier()


def r32(ap):
    return ap.bitcast(F32R)


_waitfix_n = [0]


def fix_multi_waits(nc, max_waits=1):
    """This walrus build rejects instructions carrying more than one sem
    wait. Hoist extra waits onto same-engine NoOps inserted just before the
    offending instruction (engines execute their block instructions in
    order, so waiting on a preceding NoOp is equivalent)."""
    for fn in nc.m.functions:
        for bb in fn.blocks:
            out = []
            changed = False
            for inst in bb.instructions:
                si = inst.sync_info
                if si is not None and si.on_wait and len(si.on_wait) > max_waits:
                    waits = list(si.on_wait)
                    si.on_wait = waits[:max_waits]
                    rest = waits[max_waits:]
                    while rest:
                        _waitfix_n[0] += 1
                        nop = mybir.InstNoOp(
                            name=f"waitfix-{_waitfix_n[0]}",
                            ins=[],
                            outs=[],
                            engine=inst.engine,
                            sync_info=mybir.SyncInfo(
                                on_update=[], on_wait=rest[:max_waits]
                            ),
                        )
                        nc.register_instruction(nop, overwrite=True)
                        out.append(nop)
                        rest = rest[max_waits:]
                    changed = True
                out.append(inst)
            if changed:
                bb.instructions = out


def build_attention(nc, NH, LL, DD):
    """Emit the per-core attention program into `nc`."""
    NT = LL // P          # q tiles / k chunks of 128
    NJ = LL // 512        # 512-wide k blocks per row
    HF = LL // 2          # half row for bias staging

    q = nc.dram_tensor("q", [NH, LL, DD], F32, kind="ExternalInput").ap()
    k = nc.dram_tensor("k", [NH, LL, DD], F32, kind="ExternalInput").ap()
    v = nc.dram_tensor("v", [NH, LL, DD], F32, kind="ExternalInput").ap()
    mask = nc.dram_tensor("mask", [LL, LL], I32, kind="ExternalInput").ap()
    probs = nc.dram_tensor("probs", [LL, LL], F32, kind="ExternalInput").ap()
    # [P,1] host-computed scalars: cpos = BIG*SCALE/alpha, cneg = -cpos
    cpos = nc.dram_tensor("cpos", [P, 1], F32, kind="ExternalInput").ap()
    cneg = nc.dram_tensor("cneg", [P, 1], F32, kind="ExternalInput").ap()
    # aos = alpha/SCALE  (identity scale for the bias inject)
    aos = nc.dram_tensor("aos", [P, 1], F32, kind="ExternalInput").ap()

    attn = nc.dram_tensor("attn", [NH, LL, DD], F32, kind="ExternalOutput").ap()
    p_out = nc.dram_tensor("p_out", [NH, LL, LL], F32, kind="ExternalOutput").ap()

    with PatchedTileContext(nc) as tc, ExitStack() as ctx:
        const = ctx.enter_context(tc.tile_pool(name="const", bufs=1))
        kvp = ctx.enter_context(tc.tile_pool(name="kvp", bufs=1))
        stage = ctx.enter_context(tc.tile_pool(name="stage", bufs=1))
        qtp = ctx.enter_context(tc.tile_pool(name="qtp", bufs=2))
        bias_in = ctx.enter_context(tc.tile_pool(name="bias_in", bufs=2))
        combp = ctx.enter_context(tc.tile_pool(name="combp", bufs=2))
        punp = ctx.enter_context(tc.tile_pool(name="punp", bufs=2))
        pnp = ctx.enter_context(tc.tile_pool(name="pnp", bufs=2))
        pbfp = ctx.enter_context(tc.tile_pool(name="pbfp", bufs=2))
        ptrp = ctx.enter_context(tc.tile_pool(name="ptrp", bufs=2))
        attp = ctx.enter_context(tc.tile_pool(name="attp", bufs=1))
        rsp = ctx.enter_context(tc.tile_pool(name="rsp", bufs=4))

        spsum = ctx.enter_context(tc.tile_pool(name="spsum", bufs=1, space="PSUM"))
        ptpsum = ctx.enter_context(tc.tile_pool(name="ptpsum", bufs=1, space="PSUM"))
        trpsum = ctx.enter_context(tc.tile_pool(name="trpsum", bufs=1, space="PSUM"))
        pvpsum = ctx.enter_context(tc.tile_pool(name="pvpsum", bufs=1, space="PSUM"))

        # ---- constants ----
        ident = const.tile([P, P], F32)
        make_identity(nc, ident)
        ident_bf = const.tile([P, P], BF16)
        make_identity(nc, ident_bf)
        cpos_t = const.tile([P, 1], F32)
        nc.sync.dma_start(cpos_t[:], cpos)
        cneg_t = const.tile([P, 1], F32)
        nc.sync.dma_start(cneg_t[:], cneg)
        aos_t = const.tile([P, 1], F32)
        nc.sync.dma_start(aos_t[:], aos)
        # sid = identity * (alpha/SCALE): bias-inject matmul scales comb
        sid = const.tile([P, P], F32R)
        nc.vector.tensor_scalar(sid[:], ident[:], aos_t[:, 0:1], None,
                                mybir.AluOpType.mult)

        # ---- K^T prep: kT[:, h, :]  =  k[h]^T  ([D, L] per head) ----
        kT = kvp.tile([P, NH, LL], F32R)
        for h in range(NH):
            knat = stage.tile([P, NT, DD], F32)
            nc.sync.dma_start(
                knat[:], k[h].rearrange("(c p) d -> p c d", p=P)
            )
            for c0 in range(0, NT, 4):
                g = min(4, NT - c0)
                tr = trpsum.tile([P, 512], F32)
                for c in range(g):
                    nc.tensor.transpose(
                        tr[:, c * P : (c + 1) * P],
                        knat[:, c0 + c, :],
                        ident[:],
                    )
                nc.vector.tensor_copy(
                    kT[:, h, c0 * P : (c0 + g) * P], tr[:, : g * P]
                )

        # ---- V in bf16: v_bf[:, h, c, :] = v[h][c*128:(c+1)*128, :] ----
        v_bf = kvp.tile([P, NH, NT, DD], BF16)
        nc.gpsimd.dma_start(
            v_bf[:], v.rearrange("h (c p) d -> p h c d", p=P)
        )

        # ---- main loop over q tiles ----
        for i in range(NT):
            # combined bias for this q tile (shared across heads):
            # comb = probs + cpos*mask - cpos   (cpos = BIG*SCALE/alpha)
            comb = combp.tile([P, LL], F32R)
            for half in range(2):
                sl = slice(half * HF, (half + 1) * HF)
                pr_t = bias_in.tile([P, HF], F32, tag="pr")
                nc.sync.dma_start(pr_t[:], probs[i * P : (i + 1) * P, sl])
                mk_t = bias_in.tile([P, HF], I32, tag="mk")
                nc.sync.dma_start(mk_t[:], mask[i * P : (i + 1) * P, sl])
                mb_t = bias_in.tile([P, HF], F32, tag="mb")
                nc.gpsimd.tensor_scalar(
                    mb_t[:], mk_t[:], cpos_t[:, 0:1], cneg_t[:, 0:1],
                    mybir.AluOpType.mult, mybir.AluOpType.add,
                )
                nc.gpsimd.tensor_tensor(
                    comb[:, sl], mb_t[:], pr_t[:], mybir.AluOpType.add
                )

            # qT chunks for this tile, all heads: [D, h, 128]
            qnat = qtp.tile([P, NH, DD], F32, tag="qnat")
            nc.sync.dma_start(
                qnat[:], q[:, i * P : (i + 1) * P, :].rearrange("h p d -> p h d")
            )
            qT = qtp.tile([P, NH, P], F32R, tag="qT")
            for h0 in range(0, NH, 4):
                g = min(4, NH - h0)
                tr = trpsum.tile([P, 512], F32)
                for hh in range(g):
                    nc.tensor.transpose(
                        tr[:, hh * P : (hh + 1) * P],
                        qnat[:, h0 + hh, :],
                        ident[:],
                    )
                nc.vector.tensor_copy(
                    qT[:, h0 : h0 + g, :], tr[:, : g * P]
                )

            for h in range(NH):
                # scores = inject(comb * alpha/SCALE) + q @ k^T (fp32r)
                s_ps = spsum.tile([P, LL], F32)
                for j in range(NJ):
                    js = slice(j * 512, (j + 1) * 512)
                    nc.tensor.matmul(
                        s_ps[:, js], sid[:], comb[:, js],
                        start=True, stop=False,
                    )
                    nc.tensor.matmul(
                        s_ps[:, js], qT[:, h, :], kT[:, h, js],
                        start=False, stop=True,
                    )

                # p_un = exp(SCALE * s), rs = row sums
                p_un = punp.tile([P, LL], F32)
                rs = rsp.tile([P, 1], F32, tag="rs")
                nc.scalar.activation(
                    p_un[:], s_ps[:], mybir.ActivationFunctionType.Exp,
                    bias=0.0, scale=float(SCALE), accum_out=rs[:],
                )
                rs_inv = rsp.tile([P, 1], F32, tag="rsi")
                nc.vector.reciprocal(rs_inv[:], rs[:])

                p_norm = pnp.tile([P, LL], F32)
                nc.vector.tensor_scalar(
                    p_norm[:], p_un[:], rs_inv[:, 0:1], None,
                    mybir.AluOpType.mult,
                )
                nc.sync.dma_start(p_out[h, i * P : (i + 1) * P, :], p_norm[:])

                # bf16 copy for the PV matmul
                p_bf = pbfp.tile([P, LL], BF16)
                nc.gpsimd.tensor_copy(p_bf[:], p_norm[:])

                # P^T via PE transposes (bf16), then to SBUF
                pt_ps = ptpsum.tile([P, LL], BF16)
                for c in range(NT):
                    nc.tensor.transpose(
                        pt_ps[:, c * P : (c + 1) * P],
                        p_bf[:, c * P : (c + 1) * P],
                        ident_bf[:],
                    )
                ptr = ptrp.tile([P, LL], BF16)
                nc.vector.tensor_copy(ptr[:], pt_ps[:])

                # attn tile = P @ V  (bf16 inputs, fp32 accumulate)
                pv = pvpsum.tile([P, DD], F32)
                for c in range(NT):
                    nc.tensor.matmul(
                        pv[:], ptr[:, c * P : (c + 1) * P], v_bf[:, h, c, :],
                        start=(c == 0), stop=(c == NT - 1),
                    )
                if i == 0 and h == 0:
                    attn_all = attp.tile([P, NH, NT, DD], F32)
                nc.scalar.copy(attn_all[:, h, i, :], pv[:])

        for h in range(NH):
            nc.sync.dma_start(
                attn[h].rearrange("(t p) d -> p t d", p=P), attn_all[:, h]
            )

    fix_multi_waits(nc)
    return nc


_CACHE = {}


def _get_nc(NH=HPC, LL=L, DD=D):
    key = (NH, LL, DD)
    if key not in _CACHE:
        nc = bass.Bass("TRN2", target_bir_lowering=False, debug=False)
        build_attention(nc, NH, LL, DD)
        _CACHE[key] = nc
    return _CACHE[key]


def kernel(q, k, v, mask, probs, alpha):
    q = np.asarray(q, dtype=np.float32)
    k = np.asarray(k, dtype=np.float32)
    v = np.asarray(v, dtype=np.float32)
    mask = np.ascontiguousarray(np.asarray(mask, dtype=np.int32))
    probs = np.asarray(probs, dtype=np.float32)
    alpha_f = float(np.asarray(alpha))

    if alpha_f == 0.0:
        # fold the degenerate case into the standard path: zero the probs
        # bias on the host and run with alpha=1 so masking still works.
        probs = np.zeros_like(probs)
        alpha_f = 1.0

    cpos = np.full((P, 1), BIG * SCALE / alpha_f, dtype=np.float32)
    cneg = -cpos
    aos = np.full((P, 1), alpha_f / SCALE, dtype=np.float32)

    nc = _get_nc()
    in_maps = []
    for c in range(N_CORES):
        b = c // (N_CORES // B)
        h0 = (c % (N_CORES // B)) * HPC
        in_maps.append(
            {
                "q": np.ascontiguousarray(q[b, h0 : h0 + HPC]),
                "k": np.ascontiguousarray(k[b, h0 : h0 + HPC]),
                "v": np.ascontiguousarray(v[b, h0 : h0 + HPC]),
                "mask": mask[b, 0],
                "probs": probs[b],
                "cpos": cpos,
                "cneg": cneg,
                "aos": aos,
            }
        )

    global _LAST_IN_MAPS
    _LAST_IN_MAPS = in_maps
    res = run_bass_kernel_spmd(nc, in_maps, core_ids=list(range(N_CORES)))

    attn = np.empty((B, H, L, D), dtype=np.float32)
    p = np.empty((B, H, L, L), dtype=np.float32)
    for c in range(N_CORES):
        b = c // (N_CORES // B)
        h0 = (c % (N_CORES // B)) * HPC
        attn[b, h0 : h0 + HPC] = res.results[c]["attn"]
        p[b, h0 : h0 + HPC] = res.results[c]["p_out"]
    return attn, p
